# revision 19
# baseline (speedup 1.0000x reference)
"""3-layer GCN (nn_GAT_20899310863186) on 8 TRN2 NeuronCores via Bass/Tile.

v2 strategy (evolved from v1 per trace analysis — gpsimd gather ucode and
DMA-queue pressure dominated):
  * nodes row-sharded 6250/core; edges partitioned by destination owner,
    sorted by (src-chunk, dst-window). Aggregation = dma_gather of 128-edge
    blocks (bf16 256B rows) + 0/1 one-hot matmul into a PSUM window of 128
    destination nodes.
  * one-hot matrices are PURE 0/1 and built ON-CHIP (one DVE is_equal per
    8-block chunk via stride-0 broadcast of per-edge dst columns against a
    tiled iota) instead of streaming 256B/edge of precomputed one-hots.
    coef = dis[src]*dis[dst] is separable: dis[src] folds into the gathered
    table (rows stored as dis*h, scaled on the ACT engine at write), and
    dis[dst] folds into the per-window epilogue as a per-partition scalar.
  * the activation table is AllGathered in TWO chunks (A = rows 0..3200 of
    each core, B = rest); chunk-A collective is issued mid-epilogue (after
    window 24) so collectives overlap gather/matmul work, and the chunk-B
    gather pass starts while chunk-A of the next layer is still in flight.
    Chunk tables are 25600/24400 rows — both within int16 gather range.
  * relu as tensor_scalar max (the activation-table path measured 2.8us per
    window on DVE); PSUM->acc copies moved to the idle ACT engine.
Layer algebra: L1 aggregates x@W1; L2/L3 aggregate h then apply W post-hoc
(A(h W) = (A h) W) so gather rows stay 256B.
"""

import sys

sys.path.insert(0, "/opt/trn_rl_repo")

import numpy as np
import ml_dtypes

import concourse.bacc as bacc
import concourse.mybir as mybir
import concourse.tile as tile
from concourse import library_config
from concourse.bass_utils import run_bass_kernel_spmd

BF16 = ml_dtypes.bfloat16

N, P = 50000, 8
NSH = N // P                 # 6250 nodes per core
F_IN, H1, H2, C = 256, 128, 64, 16
WIN = (NSH + 127) // 128     # 49 destination windows per core
NPAD = WIN * 128             # 6272
CHA = 3200                   # chunk-A rows per core (windows 0..24)
CHB = NSH - CHA              # 3050 rows (windows 25..48)
NA, NB = P * CHA, P * CHB    # 25600 / 24400 gather-table rows
CHUNK = 8                    # gather blocks per dma_gather call (ucode ring caps ~1024 idxs)


def _preprocess(edge_index):
    src = np.asarray(edge_index[0]).astype(np.int64)
    dst = np.asarray(edge_index[1]).astype(np.int64)
    E = src.shape[0]

    deg = (1.0 + np.bincount(dst, minlength=N)).astype(np.float32)
    dis = (1.0 / np.sqrt(deg)).astype(np.float32)

    core = dst // NSH
    dstloc = dst - core * NSH
    win = dstloc >> 7
    dstw = (dstloc & 127).astype(np.float32)
    score = src // NSH
    smod = src - score * NSH
    ch = (smod >= CHA).astype(np.int64)
    loc_src = np.where(
        ch == 0, score * CHA + smod, score * CHB + (smod - CHA)
    ).astype(np.int16)

    cnt = np.zeros((P, 2, WIN), np.int64)
    np.add.at(cnt, (core, ch, win), 1)
    B = np.maximum(1, -(-cnt.max(axis=0) // 128))       # [2, WIN] blocks
    BA, BB = int(B[0].sum()), int(B[1].sum())
    BTOT = BA + BB

    blk_base = np.zeros((2, WIN), np.int64)
    blk_base[0] = np.cumsum(B[0]) - B[0]
    blk_base[1] = np.cumsum(B[1]) - B[1]

    order = np.lexsort((win, ch, core))
    key = (core * 2 + ch) * WIN + win
    ks = key[order]
    starts = np.r_[0, np.flatnonzero(np.diff(ks)) + 1]
    gmark = np.zeros(E, np.int64)
    gmark[starts] = 1
    grp = np.cumsum(gmark) - 1
    rank = np.arange(E) - starts[grp]

    c_s, h_s, w_s = core[order], ch[order], win[order]
    slot = blk_base[h_s, w_s] * 128 + rank               # within chunk-stream

    idx_a = np.zeros((P, BA * 128), np.int16)
    idx_b = np.zeros((P, BB * 128), np.int16)
    dstw_s = np.full((P, BTOT * 128), 255.0, np.float32)  # sentinel kills pads

    lo = h_s == 0
    idx_a[c_s[lo], slot[lo]] = loc_src[order][lo]
    idx_b[c_s[~lo], slot[~lo]] = loc_src[order][~lo]
    gslot = np.where(lo, slot, BA * 128 + slot)
    dstw_s[c_s, gslot] = dstw[order]

    def wrap_idx(a):
        n = a.shape[1]
        w = a.reshape(P, n // 16, 16).transpose(0, 2, 1)
        return np.ascontiguousarray(np.tile(w, (1, 8, 1)))

    idx_a_w = wrap_idx(idx_a)
    idx_b_w = wrap_idx(idx_b)
    dc = np.ascontiguousarray(
        dstw_s.reshape(P, BTOT, 128).transpose(0, 2, 1).astype(BF16)
    )

    d2 = np.zeros((P, NPAD), np.float32)
    d2[:, :NSH] = (dis * dis).reshape(P, NSH)
    d2_w = np.ascontiguousarray(d2.reshape(P, WIN, 128).transpose(0, 2, 1))
    dv = np.zeros((P, NPAD), np.float32)
    dv[:, :NSH] = dis.reshape(P, NSH)
    dv_w = np.ascontiguousarray(dv.reshape(P, WIN, 128).transpose(0, 2, 1))

    return {
        "B": B, "idx_a": idx_a_w, "idx_b": idx_b_w,
        "dc": dc, "d2": d2_w, "dis": dv_w,
    }


def _build(B):
    f32, bf16, i16 = mybir.dt.float32, mybir.dt.bfloat16, mybir.dt.int16
    AO = mybir.AluOpType
    BA, BB = int(B[0].sum()), int(B[1].sum())
    BTOT = BA + BB

    nc = bacc.Bacc("TRN2", num_devices=P, num_swdge_queues=4, dynamic_dma_scratch_size=32768)

    xw_d = nc.dram_tensor("xw", [WIN * 2 * 128, 128], bf16, kind="ExternalInput")
    w1_d = nc.dram_tensor("W1b", [F_IN, H1], bf16, kind="ExternalInput")
    w2_d = nc.dram_tensor("W2", [H1, H2], f32, kind="ExternalInput")
    w3_d = nc.dram_tensor("W3", [H2, C], f32, kind="ExternalInput")
    b1_d = nc.dram_tensor("b1r", [128, H1], f32, kind="ExternalInput")
    b2_d = nc.dram_tensor("b2r", [128, H2], f32, kind="ExternalInput")
    b3_d = nc.dram_tensor("b3r", [128, C], f32, kind="ExternalInput")
    d2_d = nc.dram_tensor("dis2", [128, WIN], f32, kind="ExternalInput")
    dv_d = nc.dram_tensor("disv", [128, WIN], f32, kind="ExternalInput")
    id_d = nc.dram_tensor("ident", [128, 128], f32, kind="ExternalInput")
    ia_d = nc.dram_tensor("idxa", [128, BA * 8], i16, kind="ExternalInput")
    ib_d = nc.dram_tensor("idxb", [128, BB * 8], i16, kind="ExternalInput")
    dc_d = nc.dram_tensor("dcol", [128, BTOT], bf16, kind="ExternalInput")
    io_d = nc.dram_tensor("iota", [128, CHUNK * 128], bf16, kind="ExternalInput")
    out_d = nc.dram_tensor("out", [NSH, C], f32, kind="ExternalOutput")

    from contextlib import ExitStack
    with tile.TileContext(nc) as tc, ExitStack() as est:
        nc.gpsimd.load_library(library_config.mlp)
        with (
            tc.tile_pool(name="const", bufs=1) as const,
            tc.tile_pool(name="dram", bufs=1, space="DRAM") as dram,
            tc.tile_pool(name="xp", bufs=4) as xp,
            tc.tile_pool(name="gat", bufs=6) as gat,
            tc.tile_pool(name="pstr", bufs=4) as pstr,
            tc.tile_pool(name="tmp", bufs=8) as tmp,
            tc.tile_pool(name="hbp", bufs=4) as hbp,
            tc.tile_pool(name="pagg", bufs=4, space="PSUM") as pagg,
            tc.tile_pool(name="ptr", bufs=2, space="PSUM") as ptr,
            tc.tile_pool(name="pww", bufs=2, space="PSUM") as pww,
        ):
            agA_in = [dram.tile([CHA, 128], bf16, name=f"aginA{l}") for l in range(3)]
            agB_in = [dram.tile([CHB, 128], bf16, name=f"aginB{l}") for l in range(3)]
            agA_out = [
                dram.tile([NA, 128], bf16, addr_space="Shared", name=f"agoutA{l}")
                for l in range(3)
            ]
            agB_out = [
                dram.tile([NB, 128], bf16, addr_space="Shared", name=f"agoutB{l}")
                for l in range(3)
            ]

            # constants
            w1_t = const.tile([128, 2, H1], bf16)
            nc.sync.dma_start(w1_t[:], w1_d[:].rearrange("(k p) h -> p k h", p=128))
            w2_t = const.tile([128, H2], f32)
            nc.sync.dma_start(w2_t[:], w2_d[:])
            w3_t = const.tile([H2, C], f32)
            nc.sync.dma_start(w3_t[:], w3_d[:])
            b1_t = const.tile([128, H1], f32)
            nc.sync.dma_start(b1_t[:], b1_d[:])
            b2_t = const.tile([128, H2], f32)
            nc.sync.dma_start(b2_t[:], b2_d[:])
            b3_t = const.tile([128, C], f32)
            nc.sync.dma_start(b3_t[:], b3_d[:])
            d2_t = const.tile([128, WIN], f32)
            nc.sync.dma_start(d2_t[:], d2_d[:])
            dv_t = const.tile([128, WIN], f32)
            nc.sync.dma_start(dv_t[:], dv_d[:])
            idn_t = const.tile([128, 128], f32)
            nc.sync.dma_start(idn_t[:], id_d[:])
            ia_t = const.tile([128, BA * 8], i16)
            nc.sync.dma_start(ia_t[:], ia_d[:])
            ib_t = const.tile([128, BB * 8], i16)
            nc.sync.dma_start(ib_t[:], ib_d[:])
            dc_t = const.tile([128, BTOT], bf16)
            nc.sync.dma_start(dc_t[:], dc_d[:])
            io_t = const.tile([128, CHUNK * 128], bf16)
            nc.sync.dma_start(io_t[:], io_d[:])
            z_t = const.tile([128, H1], f32)
            nc.vector.memset(z_t[:], 0.0)

            A1f = const.tile([128, WIN * H1], bf16)
            h1f = const.tile([128, WIN * H1], bf16)
            h2f = const.tile([128, WIN * H2], f32)
            acc = const.tile([128, WIN * 128], f32)

            def ag_rows(w):
                return min(128, NSH - w * 128)

            def ag_write(layer, w, sb_ap):
                r = ag_rows(w)
                if w <= 24:
                    nc.sync.dma_start(
                        agA_in[layer][w * 128:w * 128 + r, :], sb_ap[:r, :]
                    )
                else:
                    o = w * 128 - CHA
                    nc.sync.dma_start(
                        agB_in[layer][o:o + r, :], sb_ap[:r, :]
                    )

            def all_gather(which, layer):
                src = agA_in[layer] if which == 0 else agB_in[layer]
                dst = agA_out[layer] if which == 0 else agB_out[layer]
                nc.gpsimd.collective_compute(
                    "AllGather",
                    AO.bypass,
                    replica_groups=[list(range(P))],
                    ins=[src.opt()],
                    outs=[dst.opt()],
                )

            # ---- phase A1 = x @ W1 on own rows (x loads batched 4 windows
            # per DMA; the sync sequencer's ~0.6us per-dispatch cost was the
            # serial head before the first gather) ----
            for wg in range(0, WIN, 4):
                nw = min(4, WIN - wg)
                xt = xp.tile([128, 8, 128], bf16, tag="xt", name="xt")
                nc.sync.dma_start(
                    xt[:, :nw * 2, :],
                    xw_d[wg * 256:(wg + nw) * 256, :]
                    .rearrange("(a p) n -> p a n", p=128),
                )
                for wi in range(nw):
                    w = wg + wi
                    ps = pagg.tile([128, H1], f32, tag="pagg", name="psA1")
                    for k in range(2):
                        nc.tensor.matmul(
                            ps[:], xt[:, wi * 2 + k, :], w1_t[:, k, :],
                            start=(k == 0), stop=(k == 1)
                        )
                    nc.vector.tensor_copy(A1f[:, w * H1:(w + 1) * H1], ps[:])
                    ab = hbp.tile([128, 128], bf16, tag="hb", name="ab")
                    nc.scalar.mul(ab[:], ps[:], dv_t[:, w:w + 1])
                    ag_write(0, w, ab)
                    if w == 24:
                        all_gather(0, 0)

            def run_pass(layer, chunk, out_cb, ag_next=False, defer_cb=None):
                src_ap = (agA_out[layer] if chunk == 0 else agB_out[layer])[:]
                idx_t = ia_t if chunk == 0 else ib_t
                bw = [int(x) for x in B[chunk]]
                pass_blocks = sum(bw)
                state = {"tile": None, "ptile": None, "base": 0, "nb": 0, "q": 0,
                         "ncall": 0}
                blk0 = 0 if chunk == 0 else BA

                def g_slice(b):
                    if state["tile"] is None or b >= state["base"] + state["nb"]:
                        if state["ncall"] == 3 and defer_cb is not None:
                            defer_cb()
                        state["ncall"] += 1
                        nb = min(CHUNK, pass_blocks - b)
                        q = state["q"]
                        t = gat.tile([128, CHUNK, 128], bf16, tag="gat", name="gt")
                        nc.gpsimd.dma_gather(
                            t[:, :nb, :], src_ap, idx_t[:, b * 8:(b + nb) * 8],
                            nb * 128, nb * 128, 128, queue_num=q,
                        )
                        pt = pstr.tile([128, CHUNK * 128], bf16, tag="pstr", name="pt")
                        po = pt[:, :nb * 128].rearrange("p (b j) -> p b j", j=128)
                        ia = io_t[:, :nb * 128].rearrange("p (b j) -> p b j", j=128)
                        da = (
                            dc_t[:, blk0 + b:blk0 + b + nb]
                            .unsqueeze(2)
                            .broadcast_to((128, nb, 128))
                        )
                        nc.vector.tensor_tensor(po, ia, da, AO.is_equal)
                        state.update(tile=t, ptile=pt, base=b, nb=nb, q=(q + 1) % 4)
                    return (state["tile"][:, b - state["base"], :],
                            state["ptile"][:, (b - state["base"]) * 128:
                                           (b - state["base"] + 1) * 128])
                pb = 0
                for w in range(WIN):
                    ps = pagg.tile([128, 128], f32, tag="pagg", name="psW")
                    for j in range(bw[w]):
                        g, pm = g_slice(pb + j)
                        nc.tensor.matmul(
                            ps[:], pm, g, start=(j == 0), stop=(j == bw[w] - 1)
                        )
                    out_cb(w, ps)
                    if ag_next and w == 26:
                        all_gather(0, layer + 1)
                    pb += bw[w]

            def lo_cb(w, ps):
                nc.scalar.copy(acc[:, w * 128:(w + 1) * 128], ps[:])

            def lo_cb64(w, ps):
                nc.scalar.copy(acc[:, w * 128:w * 128 + H2], ps[:, :H2])

            def l1_hi(w, ps):
                wsl = slice(w * H1, (w + 1) * H1)
                u = tmp.tile([128, H1], f32, tag="tA", name="u1")
                nc.vector.tensor_tensor(u[:], ps[:], acc[:, wsl], AO.add)
                nc.vector.scalar_tensor_tensor(
                    u[:], u[:], dv_t[:, w:w + 1], b1_t[:], AO.mult, AO.add
                )
                v = tmp.tile([128, H1], f32, tag="tB", name="v1")
                nc.vector.scalar_tensor_tensor(
                    v[:], A1f[:, wsl], d2_t[:, w:w + 1], u[:], AO.mult, AO.add
                )
                nc.vector.tensor_tensor(h1f[:, wsl], v[:], z_t[:], AO.max)
                hb = hbp.tile([128, 128], bf16, tag="hb", name="hb1")
                nc.scalar.mul(hb[:], h1f[:, wsl], dv_t[:, w:w + 1])
                ag_write(1, w, hb)

            def l2_hi(w, ps):
                wsl = slice(w * H1, (w + 1) * H1)
                w64 = slice(w * H2, (w + 1) * H2)
                u = tmp.tile([128, H1], f32, tag="tA", name="u2")
                nc.vector.tensor_tensor(u[:], ps[:], acc[:, wsl], AO.add)
                nc.vector.scalar_tensor_tensor(
                    u[:], u[:], dv_t[:, w:w + 1], z_t[:], AO.mult, AO.add
                )
                v = tmp.tile([128, H1], f32, tag="tB", name="v2")
                nc.vector.scalar_tensor_tensor(
                    v[:], h1f[:, wsl], d2_t[:, w:w + 1], u[:], AO.mult, AO.add
                )
                pt = ptr.tile([128, 128], f32, tag="ptr", name="pt2")
                nc.tensor.transpose(pt[:], v[:], idn_t[:])
                uT = tmp.tile([128, 128], f32, tag="tC", name="uT2")
                nc.vector.tensor_copy(uT[:], pt[:])
                pw = pww.tile([128, H2], f32, tag="pw", name="pw2")
                nc.tensor.matmul(pw[:], uT[:], w2_t[:])
                vv = tmp.tile([128, H2], f32, tag="tD", name="vv2")
                nc.vector.tensor_tensor(vv[:], pw[:], b2_t[:], AO.add)
                nc.vector.tensor_tensor(h2f[:, w64], vv[:], z_t[:, :H2], AO.max)
                hb = hbp.tile([128, 128], bf16, tag="hb", name="hb2")
                nc.vector.memset(hb[:, H2:], 0.0)
                nc.scalar.mul(hb[:, :H2], h2f[:, w64], dv_t[:, w:w + 1])
                ag_write(2, w, hb)

            def l3_hi(w, ps):
                w64 = slice(w * H2, (w + 1) * H2)
                u = tmp.tile([128, H2], f32, tag="tD", name="u3")
                nc.vector.tensor_tensor(
                    u[:], ps[:, :H2], acc[:, w * 128:w * 128 + H2], AO.add
                )
                nc.vector.scalar_tensor_tensor(
                    u[:], u[:], dv_t[:, w:w + 1], z_t[:, :H2], AO.mult, AO.add
                )
                v = tmp.tile([128, H2], f32, tag="tE", name="v3")
                nc.vector.scalar_tensor_tensor(
                    v[:], h2f[:, w64], d2_t[:, w:w + 1], u[:], AO.mult, AO.add
                )
                pt = ptr.tile([128, 128], f32, tag="ptr", name="pt3")
                nc.tensor.transpose(pt[:H2, :], v[:], idn_t[:])
                vT = tmp.tile([128, 128], f32, tag="tC", name="vT3")
                nc.vector.tensor_copy(vT[:H2, :], pt[:H2, :])
                po = pww.tile([128, C], f32, tag="pw", name="po3")
                nc.tensor.matmul(po[:], vT[:H2, :], w3_t[:])
                o = tmp.tile([128, C], f32, tag="tF", name="o3")
                nc.vector.tensor_tensor(o[:], po[:], b3_t[:], AO.add)
                r = ag_rows(w)
                nc.sync.dma_start(out_d[w * 128:w * 128 + r, :], o[:r, :])

            run_pass(0, 0, lo_cb, defer_cb=lambda: all_gather(1, 0))
            run_pass(0, 1, l1_hi, ag_next=True)
            run_pass(1, 0, lo_cb, defer_cb=lambda: all_gather(1, 1))
            run_pass(1, 1, l2_hi, ag_next=True)
            run_pass(2, 0, lo_cb64, defer_cb=lambda: all_gather(1, 2))
            run_pass(2, 1, l3_hi)

    nc.compile()
    return nc


_CACHE = {}
_IDENT = np.eye(128, dtype=np.float32)
_IOTA = np.ascontiguousarray(
    np.broadcast_to(
        np.tile(np.arange(128, dtype=np.float32), CHUNK), (128, CHUNK * 128)
    ).astype(BF16)
)


def _core_inputs(pre, x, W1, b1, W2, b2, W3, b3, c):
    xs = np.zeros((NPAD, F_IN), np.float32)
    xs[:NSH] = x[c * NSH:(c + 1) * NSH]
    xw = np.ascontiguousarray(
        xs.reshape(WIN, 128, 2, 128).transpose(0, 2, 3, 1)
        .reshape(WIN * 2 * 128, 128)
    ).astype(BF16)
    return {
        "xw": xw,
        "W1b": np.asarray(W1, np.float32).astype(BF16),
        "W2": np.asarray(W2, np.float32),
        "W3": np.asarray(W3, np.float32),
        "b1r": np.ascontiguousarray(np.broadcast_to(b1, (128, H1))).astype(np.float32),
        "b2r": np.ascontiguousarray(np.broadcast_to(b2, (128, H2))).astype(np.float32),
        "b3r": np.ascontiguousarray(np.broadcast_to(b3, (128, C))).astype(np.float32),
        "dis2": pre["d2"][c],
        "disv": pre["dis"][c],
        "ident": _IDENT,
        "idxa": pre["idx_a"][c],
        "idxb": pre["idx_b"][c],
        "dcol": pre["dc"][c],
        "iota": _IOTA,
    }


def kernel(**inputs):
    x = np.asarray(inputs["x"], dtype=np.float32)
    ei = np.asarray(inputs["edge_index"])
    W1 = np.asarray(inputs["W1"], dtype=np.float32)
    b1 = np.asarray(inputs["b1"], dtype=np.float32)
    W2 = np.asarray(inputs["W2"], dtype=np.float32)
    b2 = np.asarray(inputs["b2"], dtype=np.float32)
    W3 = np.asarray(inputs["W3"], dtype=np.float32)
    b3 = np.asarray(inputs["b3"], dtype=np.float32)

    key = hash(ei.tobytes())
    if key not in _CACHE:
        pre = _preprocess(ei)
        nc = _build(pre["B"])
        _CACHE[key] = (nc, pre)
    nc, pre = _CACHE[key]

    in_maps = [
        _core_inputs(pre, x, W1, b1, W2, b2, W3, b3, c) for c in range(P)
    ]
    res = run_bass_kernel_spmd(nc, in_maps, core_ids=list(range(P)))
    out = np.concatenate([res.results[c]["out"] for c in range(P)], axis=0)
    return np.ascontiguousarray(out, dtype=np.float32)


# revision 25
# speedup vs baseline: 1.0456x; 1.0456x over previous
"""3-layer GCN (nn_GAT_20899310863186) on 8 TRN2 NeuronCores via Bass/Tile.

v2 strategy (evolved from v1 per trace analysis — gpsimd gather ucode and
DMA-queue pressure dominated):
  * nodes row-sharded 6250/core; edges partitioned by destination owner,
    sorted by (src-chunk, dst-window). Aggregation = dma_gather of 128-edge
    blocks (bf16 256B rows) + 0/1 one-hot matmul into a PSUM window of 128
    destination nodes.
  * one-hot matrices are PURE 0/1 and built ON-CHIP (one DVE is_equal per
    8-block chunk via stride-0 broadcast of per-edge dst columns against a
    tiled iota) instead of streaming 256B/edge of precomputed one-hots.
    coef = dis[src]*dis[dst] is separable: dis[src] folds into the gathered
    table (rows stored as dis*h, scaled on the ACT engine at write), and
    dis[dst] folds into the per-window epilogue as a per-partition scalar.
  * the activation table is AllGathered in TWO chunks (A = rows 0..3200 of
    each core, B = rest); chunk-A collective is issued mid-epilogue (after
    window 24) so collectives overlap gather/matmul work, and the chunk-B
    gather pass starts while chunk-A of the next layer is still in flight.
    Chunk tables are 25600/24400 rows — both within int16 gather range.
  * relu / per-partition scaling use tensor_tensor / scalar_tensor_tensor
    forms (plain tensor_scalar measured ~2.5us per [128,128] op on DVE vs
    ~200ns for the stt/tt forms); PSUM->acc copies run on the idle ACT
    engine; x is shipped bf16 in feature-major window tiles so the A1 phase
    loads are few large contiguous DMAs (the sync sequencer's ~0.6us
    per-dispatch cost was the serial head before the first gather).
Layer algebra: L1 aggregates x@W1; L2/L3 aggregate h then apply W post-hoc
(A(h W) = (A h) W) so gather rows stay 256B.
Known hard limits (measured): dma_gather ucode accepts at most 1024 idxs
per call (1536/2048 deadlock the device) and costs ~1us fixed + ~2.7ns/idx
on the gpsimd engine; the 330 gather calls run back-to-back and are the
critical path (~1.2ms of ~1.35ms).
"""

import sys

sys.path.insert(0, "/opt/trn_rl_repo")

import numpy as np
import ml_dtypes

import concourse.bacc as bacc
import concourse.mybir as mybir
import concourse.tile as tile
from concourse import library_config
from concourse.bass_utils import run_bass_kernel_spmd

BF16 = ml_dtypes.bfloat16

N, P = 50000, 8
NSH = N // P                 # 6250 nodes per core
F_IN, H1, H2, C = 256, 128, 64, 16
WIN = (NSH + 127) // 128     # 49 destination windows per core
NPAD = WIN * 128             # 6272
CHA = 3200                   # chunk-A rows per core (windows 0..24)
CHB = NSH - CHA              # 3050 rows (windows 25..48)
NA, NB = P * CHA, P * CHB    # 25600 / 24400 gather-table rows
CHUNK = 8                    # gather blocks per dma_gather call (hard ucode cap:
                             # 1024 idxs per call; 1536 and 2048 both deadlock)


def _preprocess(edge_index):
    src = np.asarray(edge_index[0]).astype(np.int64)
    dst = np.asarray(edge_index[1]).astype(np.int64)
    E = src.shape[0]

    deg = (1.0 + np.bincount(dst, minlength=N)).astype(np.float32)
    dis = (1.0 / np.sqrt(deg)).astype(np.float32)

    core = dst // NSH
    dstloc = dst - core * NSH
    win = dstloc >> 7
    dstw = (dstloc & 127).astype(np.float32)
    score = src // NSH
    smod = src - score * NSH
    ch = (smod >= CHA).astype(np.int64)
    loc_src = np.where(
        ch == 0, score * CHA + smod, score * CHB + (smod - CHA)
    ).astype(np.int16)

    cnt = np.zeros((P, 2, WIN), np.int64)
    np.add.at(cnt, (core, ch, win), 1)
    B = np.maximum(1, -(-cnt.max(axis=0) // 128))       # [2, WIN] blocks
    BA, BB = int(B[0].sum()), int(B[1].sum())
    BTOT = BA + BB

    blk_base = np.zeros((2, WIN), np.int64)
    blk_base[0] = np.cumsum(B[0]) - B[0]
    blk_base[1] = np.cumsum(B[1]) - B[1]

    order = np.lexsort((win, ch, core))
    key = (core * 2 + ch) * WIN + win
    ks = key[order]
    starts = np.r_[0, np.flatnonzero(np.diff(ks)) + 1]
    gmark = np.zeros(E, np.int64)
    gmark[starts] = 1
    grp = np.cumsum(gmark) - 1
    rank = np.arange(E) - starts[grp]

    c_s, h_s, w_s = core[order], ch[order], win[order]
    slot = blk_base[h_s, w_s] * 128 + rank               # within chunk-stream

    idx_a = np.zeros((P, BA * 128), np.int16)
    idx_b = np.zeros((P, BB * 128), np.int16)
    dstw_s = np.full((P, BTOT * 128), 255.0, np.float32)  # sentinel kills pads

    lo = h_s == 0
    idx_a[c_s[lo], slot[lo]] = loc_src[order][lo]
    idx_b[c_s[~lo], slot[~lo]] = loc_src[order][~lo]
    gslot = np.where(lo, slot, BA * 128 + slot)
    dstw_s[c_s, gslot] = dstw[order]

    def wrap_idx(a):
        n = a.shape[1]
        w = a.reshape(P, n // 16, 16).transpose(0, 2, 1)
        return np.ascontiguousarray(np.tile(w, (1, 8, 1)))

    idx_a_w = wrap_idx(idx_a)
    idx_b_w = wrap_idx(idx_b)
    dc = np.ascontiguousarray(
        dstw_s.reshape(P, BTOT, 128).transpose(0, 2, 1).astype(BF16)
    )

    d2 = np.zeros((P, NPAD), np.float32)
    d2[:, :NSH] = (dis * dis).reshape(P, NSH)
    d2_w = np.ascontiguousarray(d2.reshape(P, WIN, 128).transpose(0, 2, 1))
    dv = np.zeros((P, NPAD), np.float32)
    dv[:, :NSH] = dis.reshape(P, NSH)
    dv_w = np.ascontiguousarray(dv.reshape(P, WIN, 128).transpose(0, 2, 1))

    return {
        "B": B, "idx_a": idx_a_w, "idx_b": idx_b_w,
        "dc": dc, "d2": d2_w, "dis": dv_w,
    }


def _build(B):
    f32, bf16, i16 = mybir.dt.float32, mybir.dt.bfloat16, mybir.dt.int16
    AO = mybir.AluOpType
    BA, BB = int(B[0].sum()), int(B[1].sum())
    BTOT = BA + BB

    nc = bacc.Bacc("TRN2", num_devices=P, num_swdge_queues=4, dynamic_dma_scratch_size=32768)

    xw_d = nc.dram_tensor("xw", [128, WIN * 2, 128], bf16, kind="ExternalInput")
    w1_d = nc.dram_tensor("W1b", [F_IN, H1], bf16, kind="ExternalInput")
    w2_d = nc.dram_tensor("W2", [H1, H2], f32, kind="ExternalInput")
    w3_d = nc.dram_tensor("W3", [H2, C], f32, kind="ExternalInput")
    b1_d = nc.dram_tensor("b1r", [128, H1], f32, kind="ExternalInput")
    b2_d = nc.dram_tensor("b2r", [128, H2], f32, kind="ExternalInput")
    b3_d = nc.dram_tensor("b3r", [128, C], f32, kind="ExternalInput")
    d2_d = nc.dram_tensor("dis2", [128, WIN], f32, kind="ExternalInput")
    dv_d = nc.dram_tensor("disv", [128, WIN], f32, kind="ExternalInput")
    id_d = nc.dram_tensor("ident", [128, 128], f32, kind="ExternalInput")
    ia_d = nc.dram_tensor("idxa", [128, BA * 8], i16, kind="ExternalInput")
    ib_d = nc.dram_tensor("idxb", [128, BB * 8], i16, kind="ExternalInput")
    dc_d = nc.dram_tensor("dcol", [128, BTOT], bf16, kind="ExternalInput")
    io_d = nc.dram_tensor("iota", [128, CHUNK * 128], bf16, kind="ExternalInput")
    out_d = nc.dram_tensor("out", [NSH, C], f32, kind="ExternalOutput")

    from contextlib import ExitStack
    with tile.TileContext(nc) as tc, ExitStack() as est:
        nc.gpsimd.load_library(library_config.mlp)
        with (
            tc.tile_pool(name="const", bufs=1) as const,
            tc.tile_pool(name="dram", bufs=1, space="DRAM") as dram,
            tc.tile_pool(name="xp", bufs=4) as xp,
            tc.tile_pool(name="gat", bufs=6) as gat,
            tc.tile_pool(name="pstr", bufs=4) as pstr,
            tc.tile_pool(name="tmp", bufs=8) as tmp,
            tc.tile_pool(name="hbp", bufs=4) as hbp,
            tc.tile_pool(name="pagg", bufs=4, space="PSUM") as pagg,
            tc.tile_pool(name="ptr", bufs=2, space="PSUM") as ptr,
            tc.tile_pool(name="pww", bufs=2, space="PSUM") as pww,
        ):
            agA_in = [dram.tile([CHA, 128], bf16, name=f"aginA{l}") for l in range(3)]
            agB_in = [dram.tile([CHB, 128], bf16, name=f"aginB{l}") for l in range(3)]
            agA_out = [
                dram.tile([NA, 128], bf16, addr_space="Shared", name=f"agoutA{l}")
                for l in range(3)
            ]
            agB_out = [
                dram.tile([NB, 128], bf16, addr_space="Shared", name=f"agoutB{l}")
                for l in range(3)
            ]

            # constants
            w1_t = const.tile([128, 2, H1], bf16)
            nc.sync.dma_start(w1_t[:], w1_d[:].rearrange("(k p) h -> p k h", p=128))
            w2_t = const.tile([128, H2], f32)
            nc.sync.dma_start(w2_t[:], w2_d[:])
            w3_t = const.tile([H2, C], f32)
            nc.sync.dma_start(w3_t[:], w3_d[:])
            b1_t = const.tile([128, H1], f32)
            nc.sync.dma_start(b1_t[:], b1_d[:])
            b2_t = const.tile([128, H2], f32)
            nc.sync.dma_start(b2_t[:], b2_d[:])
            b3_t = const.tile([128, C], f32)
            nc.sync.dma_start(b3_t[:], b3_d[:])
            d2_t = const.tile([128, WIN], f32)
            nc.sync.dma_start(d2_t[:], d2_d[:])
            dv_t = const.tile([128, WIN], f32)
            nc.sync.dma_start(dv_t[:], dv_d[:])
            idn_t = const.tile([128, 128], f32)
            nc.sync.dma_start(idn_t[:], id_d[:])
            ia_t = const.tile([128, BA * 8], i16)
            nc.sync.dma_start(ia_t[:], ia_d[:])
            ib_t = const.tile([128, BB * 8], i16)
            nc.sync.dma_start(ib_t[:], ib_d[:])
            dc_t = const.tile([128, BTOT], bf16)
            nc.sync.dma_start(dc_t[:], dc_d[:])
            io_t = const.tile([128, CHUNK * 128], bf16)
            nc.sync.dma_start(io_t[:], io_d[:])
            z_t = const.tile([128, H1], f32)
            nc.vector.memset(z_t[:], 0.0)

            A1f = const.tile([128, WIN * H1], bf16)
            h1f = const.tile([128, WIN * H1], bf16)
            h2f = const.tile([128, WIN * H2], f32)
            acc = const.tile([128, WIN * 128], f32)

            def ag_rows(w):
                return min(128, NSH - w * 128)

            def ag_write(layer, w, sb_ap):
                r = ag_rows(w)
                if w <= 24:
                    nc.sync.dma_start(
                        agA_in[layer][w * 128:w * 128 + r, :], sb_ap[:r, :]
                    )
                else:
                    o = w * 128 - CHA
                    nc.sync.dma_start(
                        agB_in[layer][o:o + r, :], sb_ap[:r, :]
                    )

            def all_gather(which, layer):
                src = agA_in[layer] if which == 0 else agB_in[layer]
                dst = agA_out[layer] if which == 0 else agB_out[layer]
                nc.gpsimd.collective_compute(
                    "AllGather",
                    AO.bypass,
                    replica_groups=[list(range(P))],
                    ins=[src.opt()],
                    outs=[dst.opt()],
                )

            # ---- phase A1 = x @ W1 on own rows (x loads batched 4 windows
            # per DMA; the sync sequencer's ~0.6us per-dispatch cost was the
            # serial head before the first gather) ----
            for wg in range(0, WIN, 4):
                nw = min(4, WIN - wg)
                xt = xp.tile([128, 8, 128], bf16, tag="xt", name="xt")
                nc.sync.dma_start(
                    xt[:, :nw * 2, :], xw_d[:, wg * 2:(wg + nw) * 2, :]
                )
                for wi in range(nw):
                    w = wg + wi
                    ps = pagg.tile([128, H1], f32, tag="pagg", name="psA1")
                    for k in range(2):
                        nc.tensor.matmul(
                            ps[:], xt[:, wi * 2 + k, :], w1_t[:, k, :],
                            start=(k == 0), stop=(k == 1)
                        )
                    nc.vector.tensor_copy(A1f[:, w * H1:(w + 1) * H1], ps[:])
                    ab = hbp.tile([128, 128], bf16, tag="hb", name="ab")
                    nc.scalar.mul(ab[:], ps[:], dv_t[:, w:w + 1])
                    ag_write(0, w, ab)
                    if w == 24:
                        all_gather(0, 0)

            def run_pass(layer, chunk, out_cb, ag_next=False, defer_cb=None):
                src_ap = (agA_out[layer] if chunk == 0 else agB_out[layer])[:]
                idx_t = ia_t if chunk == 0 else ib_t
                bw = [int(x) for x in B[chunk]]
                pass_blocks = sum(bw)
                state = {"tile": None, "ptile": None, "base": 0, "nb": 0, "q": 0,
                         "ncall": 0}
                blk0 = 0 if chunk == 0 else BA

                def g_slice(b):
                    if state["tile"] is None or b >= state["base"] + state["nb"]:
                        if state["ncall"] == 3 and defer_cb is not None:
                            defer_cb()
                        state["ncall"] += 1
                        nb = min(CHUNK, pass_blocks - b)
                        q = state["q"]
                        t = gat.tile([128, CHUNK, 128], bf16, tag="gat", name="gt")
                        nc.gpsimd.dma_gather(
                            t[:, :nb, :], src_ap, idx_t[:, b * 8:(b + nb) * 8],
                            nb * 128, nb * 128, 128, queue_num=q,
                        )
                        pt = pstr.tile([128, CHUNK * 128], bf16, tag="pstr", name="pt")
                        po = pt[:, :nb * 128].rearrange("p (b j) -> p b j", j=128)
                        ia = io_t[:, :nb * 128].rearrange("p (b j) -> p b j", j=128)
                        da = (
                            dc_t[:, blk0 + b:blk0 + b + nb]
                            .unsqueeze(2)
                            .broadcast_to((128, nb, 128))
                        )
                        nc.vector.tensor_tensor(po, ia, da, AO.is_equal)
                        state.update(tile=t, ptile=pt, base=b, nb=nb, q=(q + 1) % 4)
                    return (state["tile"][:, b - state["base"], :],
                            state["ptile"][:, (b - state["base"]) * 128:
                                           (b - state["base"] + 1) * 128])
                pb = 0
                for w in range(WIN):
                    ps = pagg.tile([128, 128], f32, tag="pagg", name="psW")
                    for j in range(bw[w]):
                        g, pm = g_slice(pb + j)
                        nc.tensor.matmul(
                            ps[:], pm, g, start=(j == 0), stop=(j == bw[w] - 1)
                        )
                    out_cb(w, ps)
                    if ag_next and w == 26:
                        all_gather(0, layer + 1)
                    pb += bw[w]

            def lo_cb(w, ps):
                nc.scalar.copy(acc[:, w * 128:(w + 1) * 128], ps[:])

            def lo_cb64(w, ps):
                nc.scalar.copy(acc[:, w * 128:w * 128 + H2], ps[:, :H2])

            def l1_hi(w, ps):
                wsl = slice(w * H1, (w + 1) * H1)
                u = tmp.tile([128, H1], f32, tag="tA", name="u1")
                nc.vector.tensor_tensor(u[:], ps[:], acc[:, wsl], AO.add)
                nc.vector.scalar_tensor_tensor(
                    u[:], u[:], dv_t[:, w:w + 1], b1_t[:], AO.mult, AO.add
                )
                v = tmp.tile([128, H1], f32, tag="tB", name="v1")
                nc.vector.scalar_tensor_tensor(
                    v[:], A1f[:, wsl], d2_t[:, w:w + 1], u[:], AO.mult, AO.add
                )
                nc.vector.tensor_tensor(h1f[:, wsl], v[:], z_t[:], AO.max)
                hb = hbp.tile([128, 128], bf16, tag="hb", name="hb1")
                nc.scalar.mul(hb[:], h1f[:, wsl], dv_t[:, w:w + 1])
                ag_write(1, w, hb)

            def l2_hi(w, ps):
                wsl = slice(w * H1, (w + 1) * H1)
                w64 = slice(w * H2, (w + 1) * H2)
                u = tmp.tile([128, H1], f32, tag="tA", name="u2")
                nc.vector.tensor_tensor(u[:], ps[:], acc[:, wsl], AO.add)
                nc.vector.scalar_tensor_tensor(
                    u[:], u[:], dv_t[:, w:w + 1], z_t[:], AO.mult, AO.add
                )
                v = tmp.tile([128, H1], f32, tag="tB", name="v2")
                nc.vector.scalar_tensor_tensor(
                    v[:], h1f[:, wsl], d2_t[:, w:w + 1], u[:], AO.mult, AO.add
                )
                pt = ptr.tile([128, 128], f32, tag="ptr", name="pt2")
                nc.tensor.transpose(pt[:], v[:], idn_t[:])
                uT = tmp.tile([128, 128], f32, tag="tC", name="uT2")
                nc.vector.tensor_copy(uT[:], pt[:])
                pw = pww.tile([128, H2], f32, tag="pw", name="pw2")
                nc.tensor.matmul(pw[:], uT[:], w2_t[:])
                vv = tmp.tile([128, H2], f32, tag="tD", name="vv2")
                nc.vector.tensor_tensor(vv[:], pw[:], b2_t[:], AO.add)
                nc.vector.tensor_tensor(h2f[:, w64], vv[:], z_t[:, :H2], AO.max)
                hb = hbp.tile([128, 128], bf16, tag="hb", name="hb2")
                nc.vector.memset(hb[:, H2:], 0.0)
                nc.scalar.mul(hb[:, :H2], h2f[:, w64], dv_t[:, w:w + 1])
                ag_write(2, w, hb)

            def l3_hi(w, ps):
                w64 = slice(w * H2, (w + 1) * H2)
                u = tmp.tile([128, H2], f32, tag="tD", name="u3")
                nc.vector.tensor_tensor(
                    u[:], ps[:, :H2], acc[:, w * 128:w * 128 + H2], AO.add
                )
                nc.vector.scalar_tensor_tensor(
                    u[:], u[:], dv_t[:, w:w + 1], z_t[:, :H2], AO.mult, AO.add
                )
                v = tmp.tile([128, H2], f32, tag="tE", name="v3")
                nc.vector.scalar_tensor_tensor(
                    v[:], h2f[:, w64], d2_t[:, w:w + 1], u[:], AO.mult, AO.add
                )
                pt = ptr.tile([128, 128], f32, tag="ptr", name="pt3")
                nc.tensor.transpose(pt[:H2, :], v[:], idn_t[:])
                vT = tmp.tile([128, 128], f32, tag="tC", name="vT3")
                nc.vector.tensor_copy(vT[:H2, :], pt[:H2, :])
                po = pww.tile([128, C], f32, tag="pw", name="po3")
                nc.tensor.matmul(po[:], vT[:H2, :], w3_t[:])
                o = tmp.tile([128, C], f32, tag="tF", name="o3")
                nc.vector.tensor_tensor(o[:], po[:], b3_t[:], AO.add)
                r = ag_rows(w)
                nc.sync.dma_start(out_d[w * 128:w * 128 + r, :], o[:r, :])

            run_pass(0, 0, lo_cb, defer_cb=lambda: all_gather(1, 0))
            run_pass(0, 1, l1_hi, ag_next=True)
            run_pass(1, 0, lo_cb, defer_cb=lambda: all_gather(1, 1))
            run_pass(1, 1, l2_hi, ag_next=True)
            run_pass(2, 0, lo_cb64, defer_cb=lambda: all_gather(1, 2))
            run_pass(2, 1, l3_hi)

    nc.compile()
    return nc


_CACHE = {}
_IDENT = np.eye(128, dtype=np.float32)
_IOTA = np.ascontiguousarray(
    np.broadcast_to(
        np.tile(np.arange(128, dtype=np.float32), CHUNK), (128, CHUNK * 128)
    ).astype(BF16)
)


def _core_inputs(pre, x, W1, b1, W2, b2, W3, b3, c):
    xs = np.zeros((NPAD, F_IN), np.float32)
    xs[:NSH] = x[c * NSH:(c + 1) * NSH]
    xw = np.ascontiguousarray(
        xs.reshape(WIN, 128, 2, 128).transpose(3, 0, 2, 1)
        .reshape(128, WIN * 2, 128)
    ).astype(BF16)
    return {
        "xw": xw,
        "W1b": np.asarray(W1, np.float32).astype(BF16),
        "W2": np.asarray(W2, np.float32),
        "W3": np.asarray(W3, np.float32),
        "b1r": np.ascontiguousarray(np.broadcast_to(b1, (128, H1))).astype(np.float32),
        "b2r": np.ascontiguousarray(np.broadcast_to(b2, (128, H2))).astype(np.float32),
        "b3r": np.ascontiguousarray(np.broadcast_to(b3, (128, C))).astype(np.float32),
        "dis2": pre["d2"][c],
        "disv": pre["dis"][c],
        "ident": _IDENT,
        "idxa": pre["idx_a"][c],
        "idxb": pre["idx_b"][c],
        "dcol": pre["dc"][c],
        "iota": _IOTA,
    }


def kernel(**inputs):
    x = np.asarray(inputs["x"], dtype=np.float32)
    ei = np.asarray(inputs["edge_index"])
    W1 = np.asarray(inputs["W1"], dtype=np.float32)
    b1 = np.asarray(inputs["b1"], dtype=np.float32)
    W2 = np.asarray(inputs["W2"], dtype=np.float32)
    b2 = np.asarray(inputs["b2"], dtype=np.float32)
    W3 = np.asarray(inputs["W3"], dtype=np.float32)
    b3 = np.asarray(inputs["b3"], dtype=np.float32)

    key = hash(ei.tobytes())
    if key not in _CACHE:
        pre = _preprocess(ei)
        nc = _build(pre["B"])
        _CACHE[key] = (nc, pre)
    nc, pre = _CACHE[key]

    in_maps = [
        _core_inputs(pre, x, W1, b1, W2, b2, W3, b3, c) for c in range(P)
    ]
    res = run_bass_kernel_spmd(nc, in_maps, core_ids=list(range(P)))
    out = np.concatenate([res.results[c]["out"] for c in range(P)], axis=0)
    return np.ascontiguousarray(out, dtype=np.float32)


# revision 28
# speedup vs baseline: 1.0769x; 1.0300x over previous
"""3-layer GCN (nn_GAT_20899310863186) on 8 TRN2 NeuronCores via Bass/Tile.

v2 strategy (evolved from v1 per trace analysis — gpsimd gather ucode and
DMA-queue pressure dominated):
  * nodes row-sharded 6250/core; edges partitioned by destination owner,
    sorted by (src-chunk, dst-window). Aggregation = dma_gather of 128-edge
    blocks (bf16 256B rows) + 0/1 one-hot matmul into a PSUM window of 128
    destination nodes.
  * one-hot matrices are PURE 0/1 and built ON-CHIP (one DVE is_equal per
    8-block chunk via stride-0 broadcast of per-edge dst columns against a
    tiled iota) instead of streaming 256B/edge of precomputed one-hots.
    coef = dis[src]*dis[dst] is separable: dis[src] folds into the gathered
    table (rows stored as dis*h, scaled on the ACT engine at write), and
    dis[dst] folds into the per-window epilogue as a per-partition scalar.
  * the activation table is AllGathered in TWO chunks (A = rows 0..3200 of
    each core, B = rest); chunk-A collective is issued mid-epilogue (after
    window 24) so collectives overlap gather/matmul work, and the chunk-B
    gather pass starts while chunk-A of the next layer is still in flight.
    Chunk tables are 25600/24400 rows — both within int16 gather range.
  * relu / per-partition scaling use tensor_tensor / scalar_tensor_tensor
    forms (plain tensor_scalar measured ~2.5us per [128,128] op on DVE vs
    ~200ns for the stt/tt forms); PSUM->acc copies run on the idle ACT
    engine; x is shipped bf16 in feature-major window tiles so the A1 phase
    loads are few large contiguous DMAs (the sync sequencer's ~0.6us
    per-dispatch cost was the serial head before the first gather).
Layer algebra: L1 aggregates x@W1; L2/L3 aggregate h then apply W post-hoc
(A(h W) = (A h) W) so gather rows stay 256B.
Known hard limits (measured): dma_gather ucode accepts at most 1024 idxs
per call (1536/2048 deadlock the device) and costs ~1us fixed + ~2.7ns/idx
on the gpsimd engine; the 330 gather calls run back-to-back and are the
critical path (~1.2ms of ~1.35ms).
"""

import sys

sys.path.insert(0, "/opt/trn_rl_repo")

import numpy as np
import ml_dtypes

import concourse.bacc as bacc
import concourse.mybir as mybir
import concourse.tile as tile
from concourse import library_config
from concourse.bass_utils import run_bass_kernel_spmd

BF16 = ml_dtypes.bfloat16

N, P = 50000, 8
NSH = N // P                 # 6250 nodes per core
F_IN, H1, H2, C = 256, 128, 64, 16
WIN = (NSH + 127) // 128     # 49 destination windows per core
NPAD = WIN * 128             # 6272
CHA = 3200                   # chunk-A rows per core (windows 0..24)
CHB = NSH - CHA              # 3050 rows (windows 25..48)
NA, NB = P * CHA, P * CHB    # 25600 / 24400 gather-table rows
CHUNK = 8                    # gather blocks per dma_gather call (hard ucode cap:
                             # 1024 idxs per call; 1536 and 2048 both deadlock)


def _preprocess(edge_index):
    src = np.asarray(edge_index[0]).astype(np.int64)
    dst = np.asarray(edge_index[1]).astype(np.int64)
    E = src.shape[0]

    deg = (1.0 + np.bincount(dst, minlength=N)).astype(np.float32)
    dis = (1.0 / np.sqrt(deg)).astype(np.float32)

    core = dst // NSH
    dstloc = dst - core * NSH
    win = dstloc >> 7
    dstw = (dstloc & 127).astype(np.float32)
    score = src // NSH
    smod = src - score * NSH
    ch = (smod >= CHA).astype(np.int64)
    loc_src = np.where(
        ch == 0, score * CHA + smod, score * CHB + (smod - CHA)
    ).astype(np.int16)

    cnt = np.zeros((P, 2, WIN), np.int64)
    np.add.at(cnt, (core, ch, win), 1)
    B = np.maximum(1, -(-cnt.max(axis=0) // 128))       # [2, WIN] blocks
    BA, BB = int(B[0].sum()), int(B[1].sum())
    BTOT = BA + BB

    blk_base = np.zeros((2, WIN), np.int64)
    blk_base[0] = np.cumsum(B[0]) - B[0]
    blk_base[1] = np.cumsum(B[1]) - B[1]

    order = np.lexsort((win, ch, core))
    key = (core * 2 + ch) * WIN + win
    ks = key[order]
    starts = np.r_[0, np.flatnonzero(np.diff(ks)) + 1]
    gmark = np.zeros(E, np.int64)
    gmark[starts] = 1
    grp = np.cumsum(gmark) - 1
    rank = np.arange(E) - starts[grp]

    c_s, h_s, w_s = core[order], ch[order], win[order]
    slot = blk_base[h_s, w_s] * 128 + rank               # within chunk-stream

    idx_a = np.zeros((P, BA * 128), np.int16)
    idx_b = np.zeros((P, BB * 128), np.int16)
    dstw_s = np.full((P, BTOT * 128), 255.0, np.float32)  # sentinel kills pads

    lo = h_s == 0
    idx_a[c_s[lo], slot[lo]] = loc_src[order][lo]
    idx_b[c_s[~lo], slot[~lo]] = loc_src[order][~lo]
    gslot = np.where(lo, slot, BA * 128 + slot)
    dstw_s[c_s, gslot] = dstw[order]

    def wrap_idx(a):
        n = a.shape[1]
        w = a.reshape(P, n // 16, 16).transpose(0, 2, 1)
        return np.ascontiguousarray(np.tile(w, (1, 8, 1)))

    idx_a_w = wrap_idx(idx_a)
    idx_b_w = wrap_idx(idx_b)
    dc = np.ascontiguousarray(
        dstw_s.reshape(P, BTOT, 128).transpose(0, 2, 1).astype(BF16)
    )

    d2 = np.zeros((P, NPAD), np.float32)
    d2[:, :NSH] = (dis * dis).reshape(P, NSH)
    d2_w = np.ascontiguousarray(d2.reshape(P, WIN, 128).transpose(0, 2, 1))
    dv = np.zeros((P, NPAD), np.float32)
    dv[:, :NSH] = dis.reshape(P, NSH)
    dv_w = np.ascontiguousarray(dv.reshape(P, WIN, 128).transpose(0, 2, 1))

    return {
        "B": B, "idx_a": idx_a_w, "idx_b": idx_b_w,
        "dc": dc, "d2": d2_w, "dis": dv_w,
    }


def _build(B):
    f32, bf16, i16 = mybir.dt.float32, mybir.dt.bfloat16, mybir.dt.int16
    AO = mybir.AluOpType
    BA, BB = int(B[0].sum()), int(B[1].sum())
    BTOT = BA + BB

    nc = bacc.Bacc("TRN2", num_devices=P, num_swdge_queues=4, dynamic_dma_scratch_size=32768)

    xw_d = nc.dram_tensor("xw", [128, WIN * 2, 128], bf16, kind="ExternalInput")
    w1_d = nc.dram_tensor("W1b", [F_IN, H1], bf16, kind="ExternalInput")
    w2_d = nc.dram_tensor("W2", [H1, H2], f32, kind="ExternalInput")
    w3_d = nc.dram_tensor("W3", [H2, C], f32, kind="ExternalInput")
    b1_d = nc.dram_tensor("b1r", [128, H1], f32, kind="ExternalInput")
    b2_d = nc.dram_tensor("b2r", [128, H2], f32, kind="ExternalInput")
    b3_d = nc.dram_tensor("b3r", [128, C], f32, kind="ExternalInput")
    d2_d = nc.dram_tensor("dis2", [128, WIN], f32, kind="ExternalInput")
    dv_d = nc.dram_tensor("disv", [128, WIN], f32, kind="ExternalInput")
    id_d = nc.dram_tensor("ident", [128, 128], f32, kind="ExternalInput")
    ia_d = nc.dram_tensor("idxa", [128, BA * 8], i16, kind="ExternalInput")
    ib_d = nc.dram_tensor("idxb", [128, BB * 8], i16, kind="ExternalInput")
    dc_d = nc.dram_tensor("dcol", [128, BTOT], bf16, kind="ExternalInput")
    io_d = nc.dram_tensor("iota", [128, CHUNK * 128], bf16, kind="ExternalInput")
    out_d = nc.dram_tensor("out", [NSH, C], f32, kind="ExternalOutput")

    from contextlib import ExitStack
    with tile.TileContext(nc) as tc, ExitStack() as est:
        nc.gpsimd.load_library(library_config.mlp)
        with (
            tc.tile_pool(name="const", bufs=1) as const,
            tc.tile_pool(name="dram", bufs=1, space="DRAM") as dram,
            tc.tile_pool(name="xp", bufs=4) as xp,
            tc.tile_pool(name="gat", bufs=6) as gat,
            tc.tile_pool(name="pstr", bufs=4) as pstr,
            tc.tile_pool(name="tmp", bufs=8) as tmp,
            tc.tile_pool(name="hbp", bufs=4) as hbp,
            tc.tile_pool(name="pagg", bufs=4, space="PSUM") as pagg,
            tc.tile_pool(name="ptr", bufs=2, space="PSUM") as ptr,
            tc.tile_pool(name="pww", bufs=2, space="PSUM") as pww,
        ):
            agA_in = [dram.tile([CHA, 128], bf16, name=f"aginA{l}") for l in range(3)]
            agB_in = [dram.tile([CHB, 128], bf16, name=f"aginB{l}") for l in range(3)]
            agA_out = [
                dram.tile([NA, 128], bf16, addr_space="Shared", name=f"agoutA{l}")
                for l in range(3)
            ]
            agB_out = [
                dram.tile([NB, 128], bf16, addr_space="Shared", name=f"agoutB{l}")
                for l in range(3)
            ]

            # constants
            w1_t = const.tile([128, 2, H1], bf16)
            nc.sync.dma_start(w1_t[:], w1_d[:].rearrange("(k p) h -> p k h", p=128))
            w2_t = const.tile([128, H2], f32)
            nc.sync.dma_start(w2_t[:], w2_d[:])
            w3_t = const.tile([H2, C], f32)
            nc.sync.dma_start(w3_t[:], w3_d[:])
            b1_t = const.tile([128, H1], f32)
            nc.sync.dma_start(b1_t[:], b1_d[:])
            b2_t = const.tile([128, H2], f32)
            nc.sync.dma_start(b2_t[:], b2_d[:])
            b3_t = const.tile([128, C], f32)
            nc.sync.dma_start(b3_t[:], b3_d[:])
            d2_t = const.tile([128, WIN], f32)
            nc.sync.dma_start(d2_t[:], d2_d[:])
            dv_t = const.tile([128, WIN], f32)
            nc.sync.dma_start(dv_t[:], dv_d[:])
            idn_t = const.tile([128, 128], f32)
            nc.sync.dma_start(idn_t[:], id_d[:])
            ia_t = const.tile([128, BA * 8], i16)
            nc.sync.dma_start(ia_t[:], ia_d[:])
            ib_t = const.tile([128, BB * 8], i16)
            nc.sync.dma_start(ib_t[:], ib_d[:])
            dc_t = const.tile([128, BTOT], bf16)
            nc.sync.dma_start(dc_t[:], dc_d[:])
            io_t = const.tile([128, CHUNK * 128], bf16)
            nc.sync.dma_start(io_t[:], io_d[:])
            z_t = const.tile([128, H1], f32)
            nc.vector.memset(z_t[:], 0.0)

            A1f = const.tile([128, WIN * H1], bf16)
            h1f = const.tile([128, WIN * H1], bf16)
            h2f = const.tile([128, WIN * H2], f32)
            acc = const.tile([128, WIN * 128], f32)

            def ag_rows(w):
                return min(128, NSH - w * 128)

            def ag_write(layer, w, sb_ap):
                r = ag_rows(w)
                if w <= 24:
                    nc.sync.dma_start(
                        agA_in[layer][w * 128:w * 128 + r, :], sb_ap[:r, :]
                    )
                else:
                    o = w * 128 - CHA
                    nc.sync.dma_start(
                        agB_in[layer][o:o + r, :], sb_ap[:r, :]
                    )

            def all_gather(which, layer):
                src = agA_in[layer] if which == 0 else agB_in[layer]
                dst = agA_out[layer] if which == 0 else agB_out[layer]
                nc.gpsimd.collective_compute(
                    "AllGather",
                    AO.bypass,
                    replica_groups=[list(range(P))],
                    ins=[src.opt()],
                    outs=[dst.opt()],
                )

            # ---- phase A1 = x @ W1 on own rows (x loads batched 4 windows
            # per DMA; the sync sequencer's ~0.6us per-dispatch cost was the
            # serial head before the first gather) ----
            for wg in range(0, WIN, 4):
                nw = min(4, WIN - wg)
                xt = xp.tile([128, 8, 128], bf16, tag="xt", name="xt")
                nc.sync.dma_start(
                    xt[:, :nw * 2, :], xw_d[:, wg * 2:(wg + nw) * 2, :]
                )
                for wi in range(nw):
                    w = wg + wi
                    ps = pagg.tile([128, H1], f32, tag="pagg", name="psA1")
                    for k in range(2):
                        nc.tensor.matmul(
                            ps[:], xt[:, wi * 2 + k, :], w1_t[:, k, :],
                            start=(k == 0), stop=(k == 1)
                        )
                    nc.vector.tensor_copy(A1f[:, w * H1:(w + 1) * H1], ps[:])
                    ab = hbp.tile([128, 128], bf16, tag="hb", name="ab")
                    nc.scalar.mul(ab[:], ps[:], dv_t[:, w:w + 1])
                    ag_write(0, w, ab)
                    if w == 24:
                        all_gather(0, 0)

            def run_pass(layer, chunk, out_cb, ag_next=False, defer_cb=None):
                src_ap = (agA_out[layer] if chunk == 0 else agB_out[layer])[:]
                idx_t = ia_t if chunk == 0 else ib_t
                bw = [int(x) for x in B[chunk]]
                pass_blocks = sum(bw)
                state = {"tile": None, "ptile": None, "base": 0, "nb": 0, "q": 0,
                         "ncall": 0}
                blk0 = 0 if chunk == 0 else BA

                def g_slice(b):
                    if state["tile"] is None or b >= state["base"] + state["nb"]:
                        if state["ncall"] == 3 and defer_cb is not None:
                            defer_cb()
                        state["ncall"] += 1
                        nb = min(CHUNK, pass_blocks - b)
                        q = state["q"]
                        t = gat.tile([128, CHUNK, 128], bf16, tag="gat", name="gt")
                        nc.gpsimd.dma_gather(
                            t[:, :nb, :], src_ap, idx_t[:, b * 8:(b + nb) * 8],
                            nb * 128, nb * 128, 128, queue_num=q,
                        )
                        pt = pstr.tile([128, CHUNK * 128], bf16, tag="pstr", name="pt")
                        po = pt[:, :nb * 128].rearrange("p (b j) -> p b j", j=128)
                        ia = io_t[:, :nb * 128].rearrange("p (b j) -> p b j", j=128)
                        da = (
                            dc_t[:, blk0 + b:blk0 + b + nb]
                            .unsqueeze(2)
                            .broadcast_to((128, nb, 128))
                        )
                        nc.vector.tensor_tensor(po, ia, da, AO.is_equal)
                        state.update(tile=t, ptile=pt, base=b, nb=nb, q=(q + 1) % 4)
                    return (state["tile"][:, b - state["base"], :],
                            state["ptile"][:, (b - state["base"]) * 128:
                                           (b - state["base"] + 1) * 128])
                pb = 0
                for w in range(WIN):
                    ps = pagg.tile([128, 128], f32, tag="pagg", name="psW")
                    for j in range(bw[w]):
                        g, pm = g_slice(pb + j)
                        nc.tensor.matmul(
                            ps[:], pm, g, start=(j == 0), stop=(j == bw[w] - 1)
                        )
                    out_cb(w, ps)
                    if ag_next and w == 26:
                        all_gather(0, layer + 1)
                    pb += bw[w]

            def lo_cb(w, ps):
                nc.scalar.copy(acc[:, w * 128:(w + 1) * 128], ps[:])

            def lo_cb64(w, ps):
                nc.scalar.copy(acc[:, w * 128:w * 128 + H2], ps[:, :H2])

            def l1_hi(w, ps):
                wsl = slice(w * H1, (w + 1) * H1)
                u = tmp.tile([128, H1], f32, tag="tA", name="u1")
                nc.vector.tensor_tensor(u[:], ps[:], acc[:, wsl], AO.add)
                nc.vector.scalar_tensor_tensor(
                    u[:], u[:], dv_t[:, w:w + 1], b1_t[:], AO.mult, AO.add
                )
                v = tmp.tile([128, H1], f32, tag="tB", name="v1")
                nc.vector.scalar_tensor_tensor(
                    v[:], A1f[:, wsl], d2_t[:, w:w + 1], u[:], AO.mult, AO.add
                )
                nc.vector.tensor_tensor(h1f[:, wsl], v[:], z_t[:], AO.max)
                hb = hbp.tile([128, 128], bf16, tag="hb", name="hb1")
                nc.scalar.mul(hb[:], h1f[:, wsl], dv_t[:, w:w + 1])
                ag_write(1, w, hb)

            def l2_hi(w, ps):
                wsl = slice(w * H1, (w + 1) * H1)
                w64 = slice(w * H2, (w + 1) * H2)
                u = tmp.tile([128, H1], f32, tag="tA", name="u2")
                nc.vector.tensor_tensor(u[:], ps[:], acc[:, wsl], AO.add)
                nc.vector.scalar_tensor_tensor(
                    u[:], u[:], dv_t[:, w:w + 1], z_t[:], AO.mult, AO.add
                )
                v = tmp.tile([128, H1], f32, tag="tB", name="v2")
                nc.vector.scalar_tensor_tensor(
                    v[:], h1f[:, wsl], d2_t[:, w:w + 1], u[:], AO.mult, AO.add
                )
                pt = ptr.tile([128, 128], f32, tag="ptr", name="pt2")
                nc.tensor.transpose(pt[:], v[:], idn_t[:])
                uT = tmp.tile([128, 128], f32, tag="tC", name="uT2")
                nc.vector.tensor_copy(uT[:], pt[:])
                pw = pww.tile([128, H2], f32, tag="pw", name="pw2")
                nc.tensor.matmul(pw[:], uT[:], w2_t[:])
                vv = tmp.tile([128, H2], f32, tag="tD", name="vv2")
                nc.vector.tensor_tensor(vv[:], pw[:], b2_t[:], AO.add)
                nc.vector.tensor_tensor(h2f[:, w64], vv[:], z_t[:, :H2], AO.max)
                hb = hbp.tile([128, 128], bf16, tag="hb", name="hb2")
                nc.vector.memset(hb[:, H2:], 0.0)
                nc.scalar.mul(hb[:, :H2], h2f[:, w64], dv_t[:, w:w + 1])
                ag_write(2, w, hb)

            def l3_hi(w, ps):
                w64 = slice(w * H2, (w + 1) * H2)
                u = tmp.tile([128, H2], f32, tag="tD", name="u3")
                nc.vector.tensor_tensor(
                    u[:], ps[:, :H2], acc[:, w * 128:w * 128 + H2], AO.add
                )
                nc.vector.scalar_tensor_tensor(
                    u[:], u[:], dv_t[:, w:w + 1], z_t[:, :H2], AO.mult, AO.add
                )
                v = tmp.tile([128, H2], f32, tag="tE", name="v3")
                nc.vector.scalar_tensor_tensor(
                    v[:], h2f[:, w64], d2_t[:, w:w + 1], u[:], AO.mult, AO.add
                )
                pt = ptr.tile([128, 128], f32, tag="ptr", name="pt3")
                nc.tensor.transpose(pt[:H2, :], v[:], idn_t[:])
                vT = tmp.tile([128, 128], f32, tag="tC", name="vT3")
                nc.vector.tensor_copy(vT[:H2, :], pt[:H2, :])
                po = pww.tile([128, C], f32, tag="pw", name="po3")
                nc.tensor.matmul(po[:], vT[:H2, :], w3_t[:])
                o = tmp.tile([128, C], f32, tag="tF", name="o3")
                nc.vector.tensor_tensor(o[:], po[:], b3_t[:], AO.add)
                r = ag_rows(w)
                nc.sync.dma_start(out_d[w * 128:w * 128 + r, :], o[:r, :])

            run_pass(0, 0, lo_cb, defer_cb=lambda: all_gather(1, 0))
            run_pass(0, 1, l1_hi, ag_next=True)
            run_pass(1, 0, lo_cb, defer_cb=lambda: all_gather(1, 1))
            run_pass(1, 1, l2_hi, ag_next=True)
            run_pass(2, 0, lo_cb64, defer_cb=lambda: all_gather(1, 2))
            run_pass(2, 1, l3_hi)

    nc.compile()
    return nc


_CACHE = {}
_IDENT = np.eye(128, dtype=np.float32)
_IOTA = np.ascontiguousarray(
    np.broadcast_to(
        np.tile(np.arange(128, dtype=np.float32), CHUNK), (128, CHUNK * 128)
    ).astype(BF16)
)


def _core_inputs(pre, x, W1, b1, W2, b2, W3, b3, c):
    xs = np.zeros((NPAD, F_IN), np.float32)
    xs[:NSH] = x[c * NSH:(c + 1) * NSH]
    xw = np.ascontiguousarray(
        xs.reshape(WIN, 128, 2, 128).transpose(3, 0, 2, 1)
        .reshape(128, WIN * 2, 128)
    ).astype(BF16)
    return {
        "xw": xw,
        "W1b": np.asarray(W1, np.float32).astype(BF16),
        "W2": np.asarray(W2, np.float32),
        "W3": np.asarray(W3, np.float32),
        "b1r": np.ascontiguousarray(np.broadcast_to(b1, (128, H1))).astype(np.float32),
        "b2r": np.ascontiguousarray(np.broadcast_to(b2, (128, H2))).astype(np.float32),
        "b3r": np.ascontiguousarray(np.broadcast_to(b3, (128, C))).astype(np.float32),
        "dis2": pre["d2"][c],
        "disv": pre["dis"][c],
        "ident": _IDENT,
        "idxa": pre["idx_a"][c],
        "idxb": pre["idx_b"][c],
        "dcol": pre["dc"][c],
        "iota": _IOTA,
    }


def kernel(**inputs):
    x = np.asarray(inputs["x"], dtype=np.float32)
    ei = np.asarray(inputs["edge_index"])
    W1 = np.asarray(inputs["W1"], dtype=np.float32)
    b1 = np.asarray(inputs["b1"], dtype=np.float32)
    W2 = np.asarray(inputs["W2"], dtype=np.float32)
    b2 = np.asarray(inputs["b2"], dtype=np.float32)
    W3 = np.asarray(inputs["W3"], dtype=np.float32)
    b3 = np.asarray(inputs["b3"], dtype=np.float32)

    key = hash(ei.tobytes())
    if key not in _CACHE:
        pre = _preprocess(ei)
        nc = _build(pre["B"])
        _CACHE[key] = (nc, pre)
    nc, pre = _CACHE[key]

    in_maps = [
        _core_inputs(pre, x, W1, b1, W2, b2, W3, b3, c) for c in range(P)
    ]
    res = run_bass_kernel_spmd(nc, in_maps, core_ids=list(range(P)))
    out = np.concatenate([res.results[c]["out"] for c in range(P)], axis=0)
    return np.ascontiguousarray(out, dtype=np.float32)
